# revision 4
# baseline (speedup 1.0000x reference)
"""RQSplineHead Trainium2 Bass kernel v2.

Per 128-row block: params = softplus(h@W.T+b)+1e-4 via relu(x)+ln(1+e^-|x|);
normalized spline tables in u-space with delta-normalized denominator
(6 chains: A, CYl, a2'=(d-dl)/W, a1'=dl, b1'=S/(dW), b2'=-S/(dW^2));
staircase chain arrays built from shared masks (8 per block) with plain
scalar_tensor_tensor ops (48 per block), split DVE/GPSIMD; batched u-space
rational evaluation per 8-block eval group; probs -> clamp -> log on ACT.

All ACT functions ({Abs, Exp, Ln, Copy, Sign, Relu/max}) live in the single
natural_log_exp_and_others table to avoid ACT table reloads.
"""

import numpy as np
from contextlib import ExitStack

import concourse.bass as bass
import concourse.mybir as mybir
import concourse.tile as tile
from concourse.masks import make_identity

f32 = mybir.dt.float32
OP = mybir.AluOpType
AF = mybir.ActivationFunctionType

B_FULL = 131072
IN_DIM = 256
NE = 129
NB = 128
NK = 9            # spline segments per row
ODIM = 27
N_CORES = 8
P = 128
G = 16            # params/table group (blocks)
B4 = 8            # eval batch (blocks)
NCH = 6           # coefficient chains

# knobs
MASKS_ON_ACT = True        # masks via ACT Sign (+-1) vs DVE is_gt (0/1)
GP_NUM, GP_DEN = 0, 48    # fraction of chain STT ops on gpsimd
EVAL_D1_ON_ACT = True      # D = Dm + 1 on ACT
CLAMP_ON_ACT = True        # max(p,1e-8) via ACT Relu trick

DEBUG_SINK = {}            # name -> dram AP; dumps emitted for sb0/group0


def _make_eval_ops():
    from concourse import dve_ops as DO
    from concourse import dve_spec as DS
    from concourse.dve_uop import DveOpSpec
    out = []
    for name, spec in [
        ("THCLAMP_ANT", DS.Spec(
            body=DS.minn(DS.maxx(DS.Src0 * DS.Src1, DS.Zero), DS.One),
            reference=lambda in0, in1, s0, s1, imm2: np.clip(
                (in0 * in1).astype(np.float32), 0.0, 1.0),
        )),
        ("T1M_ANT", DS.Spec(
            body=DS.Src0 - DS.sq(DS.Src0),
            reference=lambda in0, in1, s0, s1, imm2: in0 - in0 * in0,
        )),
    ]:
        hit = next((op for op in DO.OPS if op.name == name), None)
        if hit is not None:
            out.append(hit)
            continue
        row = max(DO._SUB_OPCODE_FOR_NAME.values()) + 1
        assert row < 0x20
        DO._SUB_OPCODE_FOR_NAME[name] = row
        shas = {}
        for ver in ("v3", "v4"):
            try:
                s = DveOpSpec(name=name, opcode=row,
                              uops=DS.lower(spec, ver=ver),
                              rd1_en=DS.Src1 in DS.spec_leaves(spec))
                shas[ver] = s.sha(ver)
            except Exception:
                pass
        assert shas, name
        op = DO.DveOp(name, spec, subdim=False, uops_sha=shas)
        DO.OPS.append(op)
        DO.CUSTOM_DVE_SPECS[name] = spec
        out.append(op)
    return out


THCLAMP_OP, T1M_OP = _make_eval_ops()


def build_rqs(ctx: ExitStack, tc: "tile.TileContext", h, W, b, out, rows):
    nc = tc.nc
    nblk = rows // P
    nsb = nblk // G
    assert nsb * G == nblk and G % B4 == 0

    const = ctx.enter_context(tc.tile_pool(name="const", bufs=1))
    psum = ctx.enter_context(tc.tile_pool(name="psum", bufs=2, space="PSUM"))
    psumg = ctx.enter_context(tc.tile_pool(name="psumg", bufs=2, space="PSUM"))
    psum1 = ctx.enter_context(tc.tile_pool(name="psum1", bufs=1, space="PSUM"))
    hpool = ctx.enter_context(tc.tile_pool(name="hpool", bufs=3))
    bs = ctx.enter_context(tc.tile_pool(name="bs", bufs=2))
    mp = ctx.enter_context(tc.tile_pool(name="mp", bufs=2))
    ev = ctx.enter_context(tc.tile_pool(name="ev", bufs=2))
    et = ctx.enter_context(tc.tile_pool(name="et", bufs=2))
    outp = ctx.enter_context(tc.tile_pool(name="outp", bufs=2))

    # ---------------- constants ----------------
    ident = const.tile([P, P], f32)
    make_identity(nc, ident)

    xi = const.tile([P, NE], mybir.dt.int32)
    nc.gpsimd.iota(xi, pattern=[[1, NE]], base=0, channel_multiplier=0)
    xconst = const.tile([P, NE], f32)
    nc.vector.tensor_scalar(xconst, xi, 1.0 / NB, None, op0=OP.mult)
    x4 = const.tile([P, B4, NE], f32)
    nc.scalar.activation(
        x4, xconst.rearrange("p (s e) -> p s e", s=1).broadcast_to([P, B4, NE]),
        AF.Copy)

    gate = const.tile([P, G, NK], f32)
    nc.vector.memset(gate, 1.0)
    nc.vector.memset(gate[:, :, 0:1], 0.0)

    ones1 = const.tile([1, P], f32)
    nc.vector.memset(ones1, 1.0)

    cbias = const.tile([P, 3], f32)
    nc.vector.memset(cbias[:, 0:1], 1.0)
    nc.vector.memset(cbias[:, 1:2], -1e-8)
    nc.vector.memset(cbias[:, 2:3], 1e-8)
    bias_one = cbias[:, 0:1]
    bias_neg8 = cbias[:, 1:2]
    bias_pos8 = cbias[:, 2:3]

    wraw = const.tile([P, IN_DIM], f32)
    nc.vector.memset(wraw, 0.0)
    nc.sync.dma_start(out=wraw[0:ODIM, :], in_=W)
    psw = psum1.tile([P, 2, P], f32)
    for k in range(2):
        nc.tensor.transpose(psw[:, k], wraw[:, k * P:(k + 1) * P], ident)
    wT = const.tile([P, 2, ODIM], f32)
    nc.scalar.copy(wT, psw[:, :, 0:ODIM])
    brow = const.tile([1, ODIM], f32)
    nc.sync.dma_start(out=brow, in_=b.rearrange("(o k) -> o k", o=1))

    pending = []

    def emit_eval(ACC, gg_blk0):
        """ACC: [P, NCH, B4, NE]; c: 0=A 1=CYl 2=W 3=H 4=dlW 5=SW0."""
        Aarr = ACC[:, 0]
        CYarr = ACC[:, 1]
        Warr = ACC[:, 2]
        Harr = ACC[:, 3]
        dlWarr = ACC[:, 4]
        SW0arr = ACC[:, 5]

        tA = et.tile([P, B4, NE], f32, tag="tA")
        tB = et.tile([P, B4, NE], f32, tag="tB")
        tC = et.tile([P, B4, NE], f32, tag="tC")
        tD = et.tile([P, B4, NE], f32, tag="tD")
        tE = et.tile([P, B4, NE], f32, tag="tE")
        u, rW, th, Hth, num = tA, tB, tC, tD, tE
        nc.vector.tensor_tensor(u, x4, Aarr, OP.subtract)
        nc.vector.reciprocal_approx_fast(rW, Warr)
        nc.vector._custom_dve(THCLAMP_OP, out=th, in0=u, in1=rW)
        nc.vector.tensor_tensor(Hth, Harr, th, OP.mult)
        Hth2 = tA  # u dead
        nc.scalar.activation(Hth2, Hth, AF.Square)
        t1m = tB  # rW dead
        nc.vector._custom_dve(T1M_OP, out=t1m, in0=th)
        # num = (dl*W*H)*t1m + (H*th)^2
        nc.vector.tensor_tensor(num, dlWarr, t1m, OP.mult)
        nc.vector.tensor_tensor(num, num, Hth2, OP.add)
        den = tD  # Hth dead
        nc.vector.tensor_tensor(den, SW0arr, t1m, OP.mult)
        nc.vector.tensor_tensor(den, den, Harr, OP.add)
        rD = tC  # th dead
        nc.vector.reciprocal_approx_fast(rD, den)
        ND = tB  # t1m dead
        nc.vector.tensor_tensor(ND, num, rD, OP.mult)

        pt = outp.tile([P, B4, NB], f32, tag="pt")
        nc.gpsimd.tensor_tensor(pt, ND[:, :, 1:NE], ND[:, :, 0:NB],
                                OP.subtract)
        ptc = outp.tile([P, B4, NB], f32, tag="ptc")
        nc.gpsimd.tensor_tensor(ptc, CYarr[:, :, 1:NE], CYarr[:, :, 0:NB],
                                OP.subtract)
        nc.gpsimd.tensor_tensor(pt, pt, ptc, OP.add)
        ot = outp.tile([P, B4, NB], f32, tag="ot")
        nc.scalar.activation(pt, pt, AF.Relu, bias=bias_neg8, scale=1.0)
        nc.scalar.activation(ot, pt, AF.Ln, bias=bias_pos8, scale=1.0)
        out_view = out[gg_blk0 * P:(gg_blk0 + B4) * P, :].rearrange(
            "(b p) n -> p b n", b=B4)
        nc.sync.dma_start(out=out_view, in_=ot)

    for sb in range(nsb):
        # ------------- phase 1: pp = h @ W.T + b (PSUM group tile) -------------
        pp = psumg.tile([P, G, ODIM], f32, tag="pp")
        for g in range(G):
            blk = sb * G + g
            r0 = blk * P
            ht = hpool.tile([P, IN_DIM], f32, tag="ht")
            nc.sync.dma_start(out=ht, in_=h[r0:r0 + P, :])
            psT = psum.tile([P, 2, P], f32, tag="psT")
            for k in range(2):
                nc.tensor.transpose(psT[:, k], ht[:, k * P:(k + 1) * P], ident)
            hT = hpool.tile([P, 2, P], f32, tag="hT")
            nc.scalar.copy(hT, psT)
            nc.tensor.matmul(pp[:, g], hT[:, 0], wT[:, 0], start=True, stop=False)
            nc.tensor.matmul(pp[:, g], hT[:, 1], wT[:, 1], start=False, stop=False)
            nc.tensor.matmul(pp[:, g], ones1, brow, start=False, stop=True)

        # ------------- phase 1b: softplus + 1e-4 (batched) -------------
        ax = bs.tile([P, G, ODIM], f32, tag="ax")
        nc.scalar.activation(ax, pp, AF.Abs)
        ex = bs.tile([P, G, ODIM], f32, tag="ex")
        nc.scalar.activation(ex, ax, AF.Exp, bias=0.0, scale=-1.0)
        l1 = bs.tile([P, G, ODIM], f32, tag="l1")
        nc.scalar.activation(l1, ex, AF.Ln, bias=1.0, scale=1.0)
        rl = bs.tile([P, G, ODIM], f32, tag="rl")
        nc.scalar.activation(rl, pp, AF.Relu)
        praw = bs.tile([P, G, ODIM], f32, tag="praw")
        nc.vector.scalar_tensor_tensor(praw, rl, 1e-4, l1, op0=OP.add, op1=OP.add)

        w_in = bs.tile([P, G, NK], f32, tag="w_in")
        nc.gpsimd.tensor_scalar(w_in, praw[:, :, 0:NK], 1.0, None, op0=OP.mult)
        h_in = bs.tile([P, G, NK], f32, tag="h_in")
        nc.gpsimd.tensor_scalar(h_in, praw[:, :, NK:2 * NK], 1.0, None,
                                op0=OP.mult)

        # ------------- phase 2: normalized tables -------------
        cx = bs.tile([P, G, NK], f32, tag="cx")
        nc.vector.tensor_tensor_scan(
            cx.rearrange("p g k -> p (g k)"),
            gate.rearrange("p g k -> p (g k)"),
            w_in.rearrange("p g k -> p (g k)"),
            0.0, op0=OP.mult, op1=OP.add)
        cy = bs.tile([P, G, NK], f32, tag="cy")
        nc.vector.tensor_tensor_scan(
            cy.rearrange("p g k -> p (g k)"),
            gate.rearrange("p g k -> p (g k)"),
            h_in.rearrange("p g k -> p (g k)"),
            0.0, op0=OP.mult, op1=OP.add)
        rSw = bs.tile([P, G], f32, tag="rSw")
        nc.vector.reciprocal_approx_fast(rSw, cx[:, :, NK - 1])
        rSh = bs.tile([P, G], f32, tag="rSh")
        nc.vector.reciprocal_approx_fast(rSh, cy[:, :, NK - 1])

        rSw_b = rSw.rearrange("p (g k) -> p g k", k=1).broadcast_to([P, G, NK])
        rSh_b = rSh.rearrange("p (g k) -> p g k", k=1).broadcast_to([P, G, NK])
        CXn = bs.tile([P, G, NK], f32, tag="CXn")
        nc.gpsimd.tensor_tensor(CXn, cx, rSw_b, OP.mult)
        CYn = bs.tile([P, G, NK], f32, tag="CYn")
        nc.gpsimd.tensor_tensor(CYn, cy, rSh_b, OP.mult)
        Wn = bs.tile([P, G, NK], f32, tag="Wn")
        nc.gpsimd.tensor_tensor(Wn, w_in, rSw_b, OP.mult)
        Hn = bs.tile([P, G, NK], f32, tag="Hn")
        nc.gpsimd.tensor_tensor(Hn, h_in, rSh_b, OP.mult)

        # TAB[p, g, c, k]: c: 0=A 1=CYl 2=W 3=H 4=dlW 5=SW0 (all bounded)
        TAB = bs.tile([P, G, NCH, NK], f32, tag="TAB")
        TABf = TAB.rearrange("p g c k -> p (g c) k")
        # A / CYl: [0, CXn[0:8]]
        nc.vector.memset(TAB[:, :, 0:2, 0:1], 0.0)
        nc.gpsimd.tensor_scalar(TAB[:, :, 0, 1:NK], CXn[:, :, 0:NK - 1],
                                1.0, None, op0=OP.mult)
        nc.gpsimd.tensor_scalar(TAB[:, :, 1, 1:NK], CYn[:, :, 0:NK - 1],
                                1.0, None, op0=OP.mult)
        # W / H copies
        nc.gpsimd.tensor_scalar(TAB[:, :, 2], Wn, 1.0, None, op0=OP.mult)
        nc.gpsimd.tensor_scalar(TAB[:, :, 3], Hn, 1.0, None, op0=OP.mult)
        # dpad: [1, d(+1e-4), 1]
        dpad = bs.tile([P, G, NK + 2], f32, tag="dpad")
        nc.vector.memset(dpad, 1.0)
        nc.gpsimd.tensor_scalar(dpad[:, :, 1:NK + 1], praw[:, :, 2 * NK:3 * NK],
                                1.0, None, op0=OP.mult)
        dl = dpad[:, :, 0:NK]
        dr = dpad[:, :, 1:NK + 1]
        # TAB4 = dl*W*H ; TAB5 = (dl+dr)*W - 2H
        dlw = bs.tile([P, G, NK], f32, tag="dlw")
        nc.vector.tensor_tensor(dlw, dl, Wn, OP.mult)
        nc.vector.tensor_tensor(TAB[:, :, 4], dlw, Hn, OP.mult)
        sdd = bs.tile([P, G, NK], f32, tag="sdd")
        nc.vector.tensor_tensor(sdd, dl, dr, OP.add)
        nc.vector.tensor_tensor(sdd, sdd, Wn, OP.mult)
        nc.vector.scalar_tensor_tensor(TAB[:, :, 5], Hn, -2.0, sdd,
                                       op0=OP.mult, op1=OP.add)

        # chain deltas + bases (0/1 masks, unhalved)
        DQ = bs.tile([P, G, NCH, NK - 1], f32, tag="DQ")
        DQf = DQ.rearrange("p g c k -> p (g c) k")
        nc.vector.tensor_tensor(DQf, TABf[:, :, 1:NK], TABf[:, :, 0:NK - 1],
                                OP.subtract)
        NTHR = bs.tile([P, G, NK - 1], f32, tag="NTHR")
        nc.vector.tensor_scalar(NTHR, CXn[:, :, 0:NK - 1], -1.0, None,
                                op0=OP.mult)
        # transposed per-eval-group bases [P, NCH, B4] (contiguous)
        BTg = bs.tile([P, G // B4, NCH, B4], f32, tag="BTg")
        for gr in range(G // B4):
            nc.vector.tensor_scalar(
                BTg[:, gr].rearrange("p c b -> p b c"),
                TAB[:, gr * B4:(gr + 1) * B4, :, 0], 1.0, None, op0=OP.mult)

        # ------------- phase 3 + 4: chains then batched eval -------------
        for g4 in range(G // B4):
            ACC = ev.tile([P, NCH, B4, NE], f32, tag="ACC")
            # seed all B4 blocks' bases in one ACT op
            bview = BTg[:, g4].rearrange(
                "p c (b e) -> p c b e", e=1)
            nc.scalar.activation(
                ACC, bview.broadcast_to([P, NCH, B4, NE]), AF.Copy)
            for bp in range(B4 // 2):
                # masks for a block pair
                msk = mp.tile([P, 2, NK - 1, NE], mybir.dt.bfloat16,
                              tag="msk")
                for half in range(2):
                    g = g4 * B4 + bp * 2 + half
                    for j in range(NK - 1):
                        nc.scalar.activation(msk[:, half, j], xconst, AF.Relu,
                                             bias=NTHR[:, g, j:j + 1], scale=1.0)
                        nc.scalar.activation(msk[:, half, j], msk[:, half, j],
                                             AF.Sign)
                if sb == 0 and g4 == 0 and bp == 0 and "msk0" in DEBUG_SINK:
                    nc.sync.dma_start(out=DEBUG_SINK["msk0"], in_=msk[:, 0])
                # chains: interleave the pair for dependency distance 12
                for j in range(NK - 1):
                    for half in range(2):
                        bb = bp * 2 + half
                        g = g4 * B4 + bb
                        for c in range(NCH):
                            nc.vector.scalar_tensor_tensor(
                                ACC[:, c, bb], msk[:, half, j],
                                DQ[:, g, c, j:j + 1],
                                ACC[:, c, bb], op0=OP.mult, op1=OP.add)
            if sb == 0 and g4 == 0 and "ACC" in DEBUG_SINK:
                nc.sync.dma_start(out=DEBUG_SINK["ACC"], in_=ACC)
            blk0 = sb * G + g4 * B4
            if pending:
                pending.pop(0)()
            pending.append(lambda A=ACC, b0=blk0: emit_eval(A, b0))

    while pending:
        pending.pop(0)()


def make_nc(rows):
    import concourse.bacc as bacc
    nc = bacc.Bacc("TRN2", target_bir_lowering=False, debug=False,
                   num_devices=N_CORES)
    h_t = nc.dram_tensor("h", [rows, IN_DIM], f32, kind="ExternalInput").ap()
    W_t = nc.dram_tensor("W", [ODIM, IN_DIM], f32, kind="ExternalInput").ap()
    b_t = nc.dram_tensor("b", [ODIM], f32, kind="ExternalInput").ap()
    out_t = nc.dram_tensor("out", [rows, NB], f32, kind="ExternalOutput").ap()
    with tile.TileContext(nc) as tc:
        with ExitStack() as ctx:
            build_rqs(ctx, tc, h_t, W_t, b_t, out_t, rows)
    nc.compile()
    return nc


_cache = {}


def kernel(h, W, b):
    h = np.ascontiguousarray(h, dtype=np.float32)
    W = np.ascontiguousarray(W, dtype=np.float32)
    b = np.ascontiguousarray(b, dtype=np.float32)
    rows = h.shape[0] // N_CORES
    key = ("nc", rows)
    if key not in _cache:
        _cache[key] = make_nc(rows)
    nc = _cache[key]
    from concourse.bass_utils import run_bass_kernel_spmd
    in_maps = [
        {"h": h[i * rows:(i + 1) * rows], "W": W, "b": b}
        for i in range(N_CORES)
    ]
    res = run_bass_kernel_spmd(nc, in_maps, core_ids=list(range(N_CORES)))
    return np.concatenate([r["out"] for r in res.results], axis=0)


if __name__ == "__main__":
    rng = np.random.default_rng(0)
    h = rng.standard_normal((B_FULL, IN_DIM), dtype=np.float32)
    W = (rng.standard_normal((ODIM, IN_DIM), dtype=np.float32) / 16.0)
    b = rng.standard_normal((ODIM,), dtype=np.float32) * 0.01
    out = kernel(h, W, b)
    print(out.shape, out.dtype, out[:2, :4])
